# revision 1
# baseline (speedup 1.0000x reference)
"""Trainium2 Bass kernel for the GRU decoder problem (nn_Decoder).

Reference computation (B=128, I=H=1024, V=50257, T=max_len=30):
    x0 = C[:,0,:] @ linC_w.T + linC_b
    h_{t+1} = GRUCell(x_t, h_t);  x_{t+1} = h_{t+1} @ lin_w.T + lin_b
    logits[:, t, :] = relu(x-side-out_t @ linres_w.T + linres_b)

Host-side algebraic folding (exact, fp64):
    - x_t is a linear function of h_t -> fold lin_w into W_ih: the recurrence
      becomes h-only with W_eff = W_ih @ lin_w (t>=1).
    - r/z gates only need gi+gh -> single matmul with W_rz = (W_eff + W_hh)[:2H].
    - the decoder output path folds to logits = relu(h_{t+1} @ W_big.T + b_big)
      with W_big = linres_w @ lin_w, b_big = linres_b + linres_w @ lin_b.
    - step 0 (which uses the C input path) is computed exactly on the host -> h1.

Device (8 NeuronCores, SPMD, no collectives):
    - the 29 remaining GRU steps are replicated on every core (the step time is
      bound by streaming the weights through the PE array, so batch-sharding
      would not reduce wall time; replication avoids any h exchange),
    - the 50257-wide vocab projection is sharded column-wise: core c computes
      vocab slice [c*6283, (c+1)*6283); outputs are concatenated on the host.
    - gates are computed batch-major (psum [128 batch, gate cols]); biases are
      folded in as a K=1 matmul against a constant ones-row; h history is kept
      transposed ([hdim, token] bf16) via per-step DMA transposes and serves as
      the stationary operand of both the next step and the projection.

All matmuls run in bf16 with fp32 PSUM accumulation (validated ~2e-3
scale-relative absmax vs the fp32 reference).
"""

import numpy as np
import ml_dtypes

import concourse.bass as bass
import concourse.tile as tile
from concourse import mybir
from concourse.bass_utils import run_bass_kernel_spmd

F32 = mybir.dt.float32
BF16 = mybir.dt.bfloat16
AF = mybir.ActivationFunctionType

B = 128
H = 1024
V = 50257
KCH = H // 128  # contraction chunks
N_CORES = 8
VS = 6283  # per-core vocab slice (8*6283 = 50264 >= V; tail zero-padded)

_bf16 = ml_dtypes.bfloat16


def patch_excess_waits(nc, maxw=1):
    """The walrus build in this container rejects >1 sync-wait attached to a
    single instruction; hoist extras into standalone EventSemaphore insts."""
    for fn in nc.m.functions:
        for bb in fn.blocks:
            new_insts = []
            for inst in bb.instructions:
                si = getattr(inst, "sync_info", None)
                if si and si.on_wait and len(si.on_wait) > maxw:
                    waits = list(si.on_wait)
                    excess, keep = waits[:-maxw], waits[-maxw:]
                    for w in excess:
                        new_insts.append(
                            mybir.InstEventSemaphore(
                                name=nc.get_next_instruction_name(),
                                opcode="EventSemaphore",
                                engine=inst.engine,
                                ins=[],
                                outs=[],
                                sync_info=mybir.SyncInfo(on_wait=[w], on_update=[]),
                            )
                        )
                    si.on_wait = keep
                new_insts.append(inst)
            bb.instructions[:] = new_insts


P1 = 3  # projection vocab 512-chunks interleaved into the GRU loop


def build_program(T=30, n_reps=0, patch=True):
    nc = bass.Bass("TRN2", target_bir_lowering=False, debug=False)

    wcat_ap = nc.dram_tensor("wcat", [KCH, 128, 4096], BF16, kind="ExternalInput").ap()
    gbias_ap = nc.dram_tensor("gbias", [128, 4096], BF16, kind="ExternalInput").ap()
    h1t_ap = nc.dram_tensor("h1t", [KCH, 128, 128], BF16, kind="ExternalInput").ap()
    h1b_ap = nc.dram_tensor("h1b", [B, H], BF16, kind="ExternalInput").ap()
    wproj_ap = nc.dram_tensor("wproj", [KCH, 128, VS], BF16, kind="ExternalInput").ap()
    bproj_ap = nc.dram_tensor("bproj", [128, VS], BF16, kind="ExternalInput").ap()
    out_ap = nc.dram_tensor("out", [T * 128, VS], F32, kind="ExternalOutput").ap()

    NCH = (VS + 511) // 512
    C1 = P1 * 512  # columns covered by the interleaved panel

    hist = None

    def emit_proj(tc, m, ch_lo, ch_hi, wp, bias_t, col_base, prp, psp):
        nc = tc.nc
        for nchunk in range(ch_lo, ch_hi):
            nw = min(512, VS - nchunk * 512)
            co = nchunk * 512
            lo = co - col_base
            ps = psp.tile([128, 512], F32, tag="pps")
            for k in range(KCH):
                nc.tensor.matmul(
                    ps[:, :nw],
                    hist[k][:, m * 128 : (m + 1) * 128],
                    wp[k][:, lo : lo + nw],
                    start=(k == 0),
                    stop=(k == KCH - 1),
                )
            ob = prp.tile([128, 512], F32, tag="ob")
            nc.vector.tensor_add(ob[:, :nw], ps[:, :nw], bias_t[:, lo : lo + nw])
            nc.scalar.activation(ob[:, :nw], ob[:, :nw], AF.Relu)
            nc.scalar.dma_start(
                out_ap[m * 128 : (m + 1) * 128, co : co + nw], ob[:, :nw]
            )

    # gate column-chunk schedule: (kind, column offset in wcat)
    GCHUNKS = (
        [("rz", j * 512) for j in range(4)]
        + [("ghn", 3072), ("ghn", 3584)]
        + [("gin", 2048), ("gin", 2560)]
    )

    def body(tc):
        nc = tc.nc
        with (
            tc.tile_pool(name="wg", bufs=1) as wgp,
            tc.tile_pool(name="gwork", bufs=1) as gw1,
            tc.tile_pool(name="gwork2", bufs=2) as gw2,
            tc.tile_pool(name="gps", bufs=4, space="PSUM") as psg,
            tc.tile_pool(name="wp1", bufs=1) as wpp1,
            tc.tile_pool(name="pwork", bufs=3) as prp,
            tc.tile_pool(name="pps", bufs=3, space="PSUM") as psp,
        ):
            wg = []
            for k in range(KCH):
                wgt = wgp.tile([128, 4096], BF16, name=f"wg{k}")
                nc.scalar.dma_start(wgt[:], wcat_ap[k])
                wg.append(wgt)
            gbias = wgp.tile([128, 4096], BF16, name="gbias")
            nc.scalar.dma_start(gbias[:], gbias_ap[:])

            wp1 = []
            for k in range(KCH):
                wpt = wpp1.tile([128, C1], BF16, name=f"wp1_{k}")
                nc.scalar.dma_start(wpt[:], wproj_ap[k][:, 0:C1])
                wp1.append(wpt)
            bias1 = wpp1.tile([128, C1], BF16, name="bias1")
            nc.scalar.dma_start(bias1[:], bproj_ap[:, 0:C1])

            for k in range(KCH):
                nc.sync.dma_start(hist[k][:, 0:128], h1t_ap[k])
            h_prev = gw2.tile([B, H], BF16, tag="hbf")
            nc.sync.dma_start(h_prev[:], h1b_ap[:])

            for t in range(1, T):
                # projection for step t-2 fills PE while step t-1's gate math runs
                if t >= 2:
                    emit_proj(tc, t - 2, 0, P1, wp1, bias1, 0, prp, psp)

                rz_s = gw1.tile([B, 2048], F32, tag="rzs")
                t1 = gw1.tile([B, H], F32, tag="t1")
                t2 = gw1.tile([B, H], F32, tag="t2")
                nt = gw1.tile([B, H], F32, tag="nt")
                h_bf = gw2.tile([B, H], BF16, tag="hbf")
                for kind, coff in GCHUNKS:
                    g = psg.tile([B, 512], F32, tag="g")
                    for k in range(KCH):
                        nc.tensor.matmul(
                            g[:],
                            hist[k][:, (t - 1) * 128 : t * 128],
                            wg[k][:, coff : coff + 512],
                            start=(k == 0),
                            stop=(k == KCH - 1),
                        )
                    gb = gw2.tile([B, 512], F32, tag="gb")
                    nc.vector.tensor_add(gb[:], g[:], gbias[:, coff : coff + 512])
                    if kind == "rz":
                        nc.scalar.activation(
                            rz_s[:, coff : coff + 512], gb[:], AF.Sigmoid
                        )
                    elif kind == "ghn":
                        jj = coff - 3072
                        nc.vector.tensor_mul(
                            t1[:, jj : jj + 512], rz_s[:, jj : jj + 512], gb[:]
                        )
                    else:  # gin
                        jj = coff - 2048
                        nc.vector.tensor_add(
                            t2[:, jj : jj + 512], t1[:, jj : jj + 512], gb[:]
                        )
                        nc.scalar.activation(
                            nt[:, jj : jj + 512], t2[:, jj : jj + 512], AF.Tanh
                        )
                        d = gw2.tile([B, 512], F32, tag="d")
                        nc.vector.tensor_sub(
                            d[:], h_prev[:, jj : jj + 512], nt[:, jj : jj + 512]
                        )
                        e = gw2.tile([B, 512], F32, tag="e")
                        nc.vector.tensor_mul(
                            e[:], rz_s[:, H + jj : H + jj + 512], d[:]
                        )
                        nc.vector.tensor_add(
                            h_bf[:, jj : jj + 512], nt[:, jj : jj + 512], e[:]
                        )
                        for kk in range(jj // 128, jj // 128 + 4):
                            nc.sync.dma_start(
                                hist[kk][:, t * 128 : (t + 1) * 128],
                                h_bf[:, kk * 128 : (kk + 1) * 128],
                                transpose=True,
                            )
                h_prev = h_bf

            # last two m-blocks' first-panel chunks while wp1 is still alive
            emit_proj(tc, T - 2, 0, P1, wp1, bias1, 0, prp, psp)
            emit_proj(tc, T - 1, 0, P1, wp1, bias1, 0, prp, psp)

        # ---------------- phase 2: remaining vocab chunks ----------------
        with (
            tc.tile_pool(name="wp2", bufs=1) as wpp2,
            tc.tile_pool(name="pwork2", bufs=4) as prp2,
            tc.tile_pool(name="pps2", bufs=3, space="PSUM") as psp2,
        ):
            C2 = VS - C1
            wp2 = []
            for k in range(KCH):
                wpt = wpp2.tile([128, C2], BF16, name=f"wp2_{k}")
                nc.scalar.dma_start(wpt[:], wproj_ap[k][:, C1:VS])
                wp2.append(wpt)
            bias2 = wpp2.tile([128, C2], BF16, name="bias2")
            nc.scalar.dma_start(bias2[:], bproj_ap[:, C1:VS])

            for m in range(T):
                emit_proj(tc, m, P1, NCH, wp2, bias2, C1, prp2, psp2)

    with tile.TileContext(nc) as tc:
        with tc.tile_pool(name="hist", bufs=1) as histp:
            hist = [
                histp.tile([128, T * 128], BF16, name=f"hist{k}") for k in range(KCH)
            ]
            if n_reps > 0:
                with tc.For_i(0, n_reps, 1):
                    body(tc)
            else:
                body(tc)

    if patch:
        patch_excess_waits(nc)
    return nc


# ---------------- host side ----------------


def fold_weights(W_ih, W_hh, b_ih, b_hh, lin_w, lin_b, linres_w, linres_b):
    W_ih = W_ih.astype(np.float64)
    W_hh = W_hh.astype(np.float64)
    lin_w64 = lin_w.astype(np.float64)
    lin_b64 = lin_b.astype(np.float64)
    W_eff = W_ih @ lin_w64
    b_eff = b_ih.astype(np.float64) + W_ih @ lin_b64
    W_rz = (W_eff + W_hh)[: 2 * H]
    b_rz = b_eff[: 2 * H] + b_hh.astype(np.float64)[: 2 * H]
    W_in = W_eff[2 * H :]
    b_in = b_eff[2 * H :]
    W_hn = W_hh[2 * H :]
    b_hn = b_hh.astype(np.float64)[2 * H :]
    W_big = linres_w.astype(np.float32) @ lin_w.astype(np.float32)
    b_big = linres_b.astype(np.float64) + linres_w.astype(np.float64) @ lin_b64
    return W_rz, b_rz, W_in, b_in, W_hn, b_hn, W_big, b_big


def host_step0(C, init_hidden, W_ih, W_hh, b_ih, b_hh, linC_w, linC_b):
    h0 = init_hidden[0].astype(np.float64)
    x0 = C[:, 0, :].astype(np.float64) @ linC_w.astype(np.float64).T + linC_b.astype(
        np.float64
    )
    gi = x0 @ W_ih.astype(np.float64).T + b_ih.astype(np.float64)
    gh = h0 @ W_hh.astype(np.float64).T + b_hh.astype(np.float64)
    r = 1.0 / (1.0 + np.exp(-(gi[:, :H] + gh[:, :H])))
    z = 1.0 / (1.0 + np.exp(-(gi[:, H : 2 * H] + gh[:, H : 2 * H])))
    n = np.tanh(gi[:, 2 * H :] + r * gh[:, 2 * H :])
    return (1.0 - z) * n + z * h0


def make_input_maps(inputs):
    W_rz, b_rz, W_in, b_in, W_hn, b_hn, W_big, b_big = fold_weights(
        np.asarray(inputs["W_ih"]),
        np.asarray(inputs["W_hh"]),
        np.asarray(inputs["b_ih"]),
        np.asarray(inputs["b_hh"]),
        np.asarray(inputs["lin_w"]),
        np.asarray(inputs["lin_b"]),
        np.asarray(inputs["linres_w"]),
        np.asarray(inputs["linres_b"]),
    )
    h1 = host_step0(
        np.asarray(inputs["C"]),
        np.asarray(inputs["init_hidden"]),
        np.asarray(inputs["W_ih"]),
        np.asarray(inputs["W_hh"]),
        np.asarray(inputs["b_ih"]),
        np.asarray(inputs["b_hh"]),
        np.asarray(inputs["linC_w"]),
        np.asarray(inputs["linC_b"]),
    )

    Wcat = np.concatenate([W_rz.T, W_in.T, W_hn.T], axis=1)  # [H, 4096]
    wcat = np.ascontiguousarray(Wcat.reshape(KCH, 128, 4096).astype(np.float32)).astype(
        _bf16
    )
    gbias_row = np.concatenate([b_rz, b_in, b_hn]).astype(np.float32)
    gbias = np.ascontiguousarray(
        np.broadcast_to(gbias_row[None, :], (128, 4096))
    ).astype(_bf16)
    h1_bf = h1.astype(np.float32).astype(_bf16)
    h1t = np.ascontiguousarray(h1_bf.astype(np.float32).T.reshape(KCH, 128, 128)).astype(
        _bf16
    )

    WbT = W_big.T.astype(np.float32)  # [H, V]
    in_maps = []
    for c in range(N_CORES):
        v0 = c * VS
        v1 = min(V, v0 + VS)
        wslice = np.zeros((H, VS), np.float32)
        wslice[:, : v1 - v0] = WbT[:, v0:v1]
        wproj = np.ascontiguousarray(wslice.reshape(KCH, 128, VS)).astype(_bf16)
        bslice = np.zeros((VS,), np.float32)
        bslice[: v1 - v0] = b_big[v0:v1].astype(np.float32)
        bproj = np.ascontiguousarray(
            np.broadcast_to(bslice[None, :], (128, VS))
        ).astype(_bf16)
        in_maps.append(
            {
                "wcat": wcat,
                "gbias": gbias,
                "h1t": h1t,
                "h1b": h1_bf,
                "wproj": wproj,
                "bproj": bproj,
            }
        )
    return in_maps


def assemble_output(results, T):
    full = np.empty((B, T, V), np.float32)
    for c in range(N_CORES):
        v0 = c * VS
        v1 = min(V, v0 + VS)
        oc = results[c]["out"].reshape(T, B, VS)[:, :, : v1 - v0]
        full[:, :, v0:v1] = oc.transpose(1, 0, 2)
    return full


_PROGRAMS = {}


def _get_program(T, n_reps=0):
    key = (T, n_reps)
    if key not in _PROGRAMS:
        _PROGRAMS[key] = build_program(T=T, n_reps=n_reps)
    return _PROGRAMS[key]


def kernel(**inputs):
    T = int(inputs["max_len"])
    in_maps = make_input_maps(inputs)
    nc = _get_program(T)
    br = run_bass_kernel_spmd(nc, in_maps, list(range(N_CORES)))
    return assemble_output(br.results, T)


def measure_hw_exec_ns(inputs, reps=64):
    """Estimate per-invocation HW time by differencing a repeat-loop program
    against the single-shot program (same inputs)."""
    import time as _time

    T = int(inputs["max_len"])
    in_maps = make_input_maps(inputs)

    def run_k(n_reps, warm=1, meas=3):
        nc = _get_program(T, n_reps)
        ts = []
        for i in range(warm + meas):
            t0 = _time.perf_counter()
            run_bass_kernel_spmd(nc, in_maps, list(range(N_CORES)))
            t1 = _time.perf_counter()
            if i >= warm:
                ts.append(t1 - t0)
        return min(ts)

    t1 = run_k(0)
    tk = run_k(reps)
    return (tk - t1) / (reps - 1) * 1e9

